# revision 1
# baseline (speedup 1.0000x reference)
"""Trainium2 Bass kernel v2: MultiHeadAttention with rel-pos bias via
one-hot-plane matmuls in an (h, k16) packed layout.

Problem: B=4, S=2048, D=256, H=8, d_k=32.  8 cores = (batch, query-half);
each core: 8 heads x 1024 q x 2048 k.

Core idea: emb row 9 is zero (padding_idx), so the per-head bias
  bias[k,q] = emb_fwd[rpF[k,q],h] + emb_bwd[rpN[k,q],h]
needs only 9 one-hot planes per direction.  With scores in a packed
layout p = h*16+k16 (k-super-tiles ST of 16 rows), the bias for ALL
heads is 3 PE matmuls per (ST, q-chunk) over head-independent one-hot
planes G[(slot,k16), q], with host-built coefficient matrices
LHS[(slot,k16),(h,k16)] = emb[v_slot, h] * [k16==k16'].

Pipeline per (ST, qc=512):
  psumS = KBD_g0^T Q_g0 + KBD_g1^T Q_g1        (block-diag K, 2 mm)
        + LHS1^T G1 + LHS2^T G2 + LHS3^T G3    (bias, 3 mm)
  attn  = exp(psumS)  (ACT)                     [no max-sub; logits bounded]
  psumA += VrepA^T attn ; psumB += VrepB^T attn (dv 0-15 / 16-31 sections)
  psumD += Mden^T attn                          (denominator)
After all ST: recip(psumD) -> rb broadcast matmul -> OA = psumA * rb,
OB = psumB * rb -> out-proj with host-reordered Wp rows; bv folded into
bp on host (softmax rows sum to 1).
"""

import sys

if "/opt/trn_rl_repo" not in sys.path:
    sys.path.insert(0, "/opt/trn_rl_repo")

import numpy as np

import concourse.bass as bass
import concourse.mybir as mybir
from concourse import bacc
from concourse.tile import TileContext
from concourse.bass_utils import run_bass_kernel_spmd

B, S, D, H = 4, 2048, 256, 8
D_K = D // H
QH = S // 2
N_CORES = 8
NST = S // 16          # 128 k-super-tiles
KT_TILES = S // 128    # 16 (for rp replication DMAs: 8 STs each)
FP32 = mybir.dt.float32
FP16 = mybir.dt.float16
BF16 = mybir.dt.bfloat16

# plane slots: tile1 = F v0..7 ; tile2 = [F v8, N v0..6] ; tile3 = [N v7, N v8]
T1V = list(range(8))
T2V = [8, 0, 1, 2, 3, 4, 5, 6]
T3V = [7, 8]


def _build():
    nc = bacc.Bacc("TRN2", target_bir_lowering=False, debug=False)

    qT = nc.dram_tensor("qT", [D, QH], FP16, kind="ExternalInput").ap()
    kT = nc.dram_tensor("kT", [D, S], FP16, kind="ExternalInput").ap()
    vT = nc.dram_tensor("vT", [D, S], FP16, kind="ExternalInput").ap()
    rpF = nc.dram_tensor("rpF", [S, QH], BF16, kind="ExternalInput").ap()
    rpN = nc.dram_tensor("rpN", [S, QH], BF16, kind="ExternalInput").ap()
    wqT = nc.dram_tensor("wqT", [D, D], FP16, kind="ExternalInput").ap()
    wkT = nc.dram_tensor("wkT", [D, D], FP16, kind="ExternalInput").ap()
    wvT = nc.dram_tensor("wvT", [D, D], FP16, kind="ExternalInput").ap()
    wpA = nc.dram_tensor("wpA", [128, D], FP16, kind="ExternalInput").ap()
    wpB = nc.dram_tensor("wpB", [128, D], FP16, kind="ExternalInput").ap()
    bqs = nc.dram_tensor("bqs", [128, 2], FP32, kind="ExternalInput").ap()
    bks = nc.dram_tensor("bks", [128, 2], FP32, kind="ExternalInput").ap()
    bps = nc.dram_tensor("bps", [128, 2], FP32, kind="ExternalInput").ap()
    lhs1 = nc.dram_tensor("lhs1", [128, 128], FP16, kind="ExternalInput").ap()
    lhs2 = nc.dram_tensor("lhs2", [128, 128], FP16, kind="ExternalInput").ap()
    lhs3 = nc.dram_tensor("lhs3", [32, 128], FP16, kind="ExternalInput").ap()
    vc1 = nc.dram_tensor("vc1", [128, 1], FP32, kind="ExternalInput").ap()
    vc2 = nc.dram_tensor("vc2", [128, 1], FP32, kind="ExternalInput").ap()
    vc3 = nc.dram_tensor("vc3", [32, 1], FP32, kind="ExternalInput").ap()
    mvf = nc.dram_tensor("mvf", [128, 128], FP16, kind="ExternalInput").ap()
    rep16 = nc.dram_tensor("rep16", [16, 128], BF16, kind="ExternalInput").ap()
    rep128 = nc.dram_tensor("rep128", [128, 1024], FP16, kind="ExternalInput").ap()
    mkb0 = nc.dram_tensor("mkb0", [128, 1024], FP16, kind="ExternalInput").ap()
    mkb1 = nc.dram_tensor("mkb1", [128, 1024], FP16, kind="ExternalInput").ap()
    mden = nc.dram_tensor("mden", [128, 8], FP16, kind="ExternalInput").ap()
    lden = nc.dram_tensor("lden", [8, 128], FP32, kind="ExternalInput").ap()
    outT = nc.dram_tensor("outT", [D, QH], FP32, kind="ExternalOutput").ap()

    with TileContext(nc) as tc:
        _emit(nc, tc, locals())
    nc.compile()
    return nc


def _emit(nc, tc, t):
    qT, kT, vT, rpF, rpN = t["qT"], t["kT"], t["vT"], t["rpF"], t["rpN"]
    wqT, wkT, wvT = t["wqT"], t["wkT"], t["wvT"]
    wpA, wpB = t["wpA"], t["wpB"]
    bqs, bks, bps = t["bqs"], t["bks"], t["bps"]
    lhs1, lhs2, lhs3 = t["lhs1"], t["lhs2"], t["lhs3"]
    vc1, vc2, vc3 = t["vc1"], t["vc2"], t["vc3"]
    mvf, rep16 = t["mvf"], t["rep16"]
    rep128, mkb0, mkb1 = t["rep128"], t["mkb0"], t["mkb1"]
    mden, lden = t["mden"], t["lden"]
    outT = t["outT"]
    Exp = mybir.ActivationFunctionType.Exp
    Ident = mybir.ActivationFunctionType.Identity
    AOT = mybir.AluOpType

    import contextlib
    ctx = contextlib.ExitStack()
    with ctx:
        singles = ctx.enter_context(tc.tile_pool(name="singles", bufs=1))
        stage = ctx.enter_context(tc.tile_pool(name="stage", bufs=3))
        repp = ctx.enter_context(tc.tile_pool(name="rep", bufs=2))
        gp = ctx.enter_context(tc.tile_pool(name="g", bufs=3))
        kbdp = ctx.enter_context(tc.tile_pool(name="kbd", bufs=3))
        vrp = ctx.enter_context(tc.tile_pool(name="vr", bufs=3))
        attp = ctx.enter_context(tc.tile_pool(name="att", bufs=4))
        # PSUM budget (8 banks): psS "scores" x3 rotating + psA/psB x2qc + psDD
        psS = ctx.enter_context(tc.tile_pool(name="psS", bufs=3, space="PSUM"))
        psO = ctx.enter_context(tc.tile_pool(name="psO", bufs=1, space="PSUM"))

        # ---- constants ----
        c_sb = {}
        for name, ap, shp, dt in (
            ("lhs1", lhs1, [128, 128], FP16), ("lhs2", lhs2, [128, 128], FP16),
            ("lhs3", lhs3, [32, 128], FP16), ("vc1", vc1, [128, 1], FP32),
            ("vc2", vc2, [128, 1], FP32), ("vc3", vc3, [32, 1], FP32),
            ("mvf", mvf, [128, 128], FP16), ("rep16", rep16, [16, 128], BF16),
            ("rep128", rep128, [128, 1024], FP16),
            ("mkb0", mkb0, [128, 1024], FP16), ("mkb1", mkb1, [128, 1024], FP16),
            ("mden", mden, [128, 8], FP16), ("lden", lden, [8, 128], FP32),
            ("bqs", bqs, [128, 2], FP32), ("bks", bks, [128, 2], FP32),
            ("bps", bps, [128, 2], FP32),
        ):
            tl = singles.tile(shp, dt, name=name, tag=name)
            nc.sync.dma_start(out=tl, in_=ap)
            c_sb[name] = tl

        # ---- weights ----
        w_sb = {}
        for name, ap in (("wq", wqT), ("wk", wkT), ("wv", wvT)):
            for g in range(2):
                tl = singles.tile([128, D], FP16, name=f"w_{name}{g}", tag=f"w_{name}{g}")
                nc.sync.dma_start(out=tl, in_=ap[g * 128:(g + 1) * 128, :])
                w_sb[name, g] = tl
        for name, ap in (("wpA", wpA), ("wpB", wpB)):
            tl = singles.tile([128, D], FP16, name=name, tag=name)
            nc.sync.dma_start(out=tl, in_=ap)
            w_sb[name] = tl

        # ---- Q/K projections -> QTs [2][128,QH] fp16, KTs [2][128,S] fp16 ----
        QTs = [singles.tile([128, QH], FP16, name=f"QTs{g}", tag=f"QTs{g}") for g in range(2)]
        for dst, src_dram, wname, bname, width in (
            (QTs, qT, "wq", "bqs", QH),
        ):
            for c0 in range(0, width, 512):
                xc = [stage.tile([128, 512], FP16, name=f"xT{dg}", tag=f"xT{dg}")
                      for dg in range(2)]
                for dg in range(2):
                    nc.sync.dma_start(
                        out=xc[dg], in_=src_dram[dg * 128:(dg + 1) * 128, c0:c0 + 512])
                for g in range(2):
                    ps = psS.tile([128, 512], FP32, name="proj", tag="scores")
                    for dg in range(2):
                        nc.tensor.matmul(
                            ps, w_sb[wname, dg][:, g * 128:(g + 1) * 128], xc[dg],
                            start=(dg == 0), stop=(dg == 1))
                    nc.scalar.activation(
                        dst[g][:, c0:c0 + 512], ps, Ident,
                        bias=c_sb[bname][:, g:g + 1])



        # ---- attnV output accumulators (per qc): A, B + combined denom ----
        psA = [psO.tile([128, 512], FP32, name=f"psA{qc}", tag=f"psA{qc}") for qc in range(2)]
        psB = [psO.tile([128, 512], FP32, name=f"psB{qc}", tag=f"psB{qc}") for qc in range(2)]
        psDD = psO.tile([40, 512], FP32, name="psDD", tag="psDD")

        # ---- main loop over ktiles (rp replication granularity) and STs ----
        for kt in range(KT_TILES):
            # replicated rp tiles: [slot*16 partitions, (st8, q)]
            t1 = repp.tile([128, 8 * QH], BF16, name="t1", tag="t1")
            t2 = repp.tile([128, 8 * QH], BF16, name="t2", tag="t2")
            t3 = repp.tile([32, 8 * QH], BF16, name="t3", tag="t3")
            base = kt * 128 * QH
            src3 = lambda src: bass.AP(
                tensor=src.tensor, offset=src.offset + base,
                ap=[[QH, 16], [16 * QH, 8], [1, QH]])
            for j, (tile, p0, src) in enumerate(
                [(t1, 16 * j2, rpF) for j2 in range(8)]
                + [(t2, 0, rpF)] + [(t2, 16 + 16 * j2, rpN) for j2 in range(7)]
                + [(t3, 16 * j2, rpN) for j2 in range(2)]
            ):
                eng = (nc.sync, nc.gpsimd, nc.scalar, nc.gpsimd)[j % 4]
                eng.dma_start(out=tile[p0:p0 + 16, :], in_=src3(src))

            # per-ktile K/V input chunks + K-major projections [k128, dout256]
            kvc = {}
            for nm, src in (("k", kT), ("v", vT)):
                for dg in range(2):
                    cchunk = stage.tile([128, 128], FP16, name=f"{nm}c{dg}",
                                        tag=f"{nm}c{dg}")
                    nc.sync.dma_start(
                        out=cchunk,
                        in_=src[dg * 128:(dg + 1) * 128,
                                kt * 128:(kt + 1) * 128])
                    kvc[nm, dg] = cchunk
            vnat_st = []
            for st8 in range(8):
                vps = psS.tile([128, 512], FP32, name="vnatp", tag="scores")
                for dg in range(2):
                    nc.tensor.matmul(
                        vps[0:16, 0:256],
                        kvc["v", dg][:, st8 * 16:(st8 + 1) * 16],
                        w_sb["wv", dg],
                        start=(dg == 0), stop=(dg == 1))
                vt = repp.tile([16, 256], BF16, name=f"vnat{st8}", tag=f"vnat{st8}")
                nc.scalar.activation(vt, vps[0:16, 0:256], Ident)
                vnat_st.append(vt)
            ktt_ps = psS.tile([128, 512], FP32, name="ktt", tag="scores")
            for dg in range(2):
                nc.tensor.matmul(
                    ktt_ps[:, 0:256],
                    kvc["k", dg], w_sb["wk", dg],
                    start=(dg == 0), stop=(dg == 1))
            ktt = repp.tile([128, 256], FP16, name="ktt", tag="kttsb")
            nc.scalar.activation(ktt, ktt_ps[:, 0:256], Ident)
            # KBD_big[d, (st8,h,k16)] = KT[d, k] * mask(d,h), via PE replication
            kbd_big = []
            for g in range(2):
                kbb = repp.tile([128, 1024], FP16, name=f"kbdb{g}", tag=f"kbdb{g}")
                for ch in range(2):
                    kps = psS.tile([128, 512], FP32, name="kbdps", tag="scores")
                    nc.tensor.matmul(
                        kps, ktt[:, g * 128:(g + 1) * 128],
                        c_sb["rep128"][:, ch * 512:(ch + 1) * 512],
                        start=True, stop=True)
                    krep = attp.tile([128, 512], FP16, name="krep", tag="krep")
                    nc.scalar.copy(krep, kps)
                    nc.vector.tensor_tensor(
                        out=kbb[:, ch * 512:(ch + 1) * 512], in0=krep,
                        in1=c_sb[f"mkb{g}"][:, ch * 512:(ch + 1) * 512],
                        op=AOT.mult)
                kbd_big.append(kbb)

            for st8 in range(8):
                ST = kt * 8 + st8
                k0 = ST * 16
                qsl = slice(st8 * QH, (st8 + 1) * QH)
                ssl = slice(st8 * 128, (st8 + 1) * 128)

                # one-hot planes
                g1 = gp.tile([128, QH], BF16, name="g1", tag="g1")
                g2 = gp.tile([128, QH], BF16, name="g2", tag="g2")
                g3 = gp.tile([32, QH], BF16, name="g3", tag="g3")
                nc.vector.tensor_scalar(
                    out=g1, in0=t1[:, qsl], scalar1=c_sb["vc1"][:, 0:1],
                    scalar2=None, op0=AOT.is_equal)
                nc.vector.tensor_scalar(
                    out=g2, in0=t2[:, qsl], scalar1=c_sb["vc2"][:, 0:1],
                    scalar2=None, op0=AOT.is_equal)
                nc.vector.tensor_scalar(
                    out=g3, in0=t3[:, qsl], scalar1=c_sb["vc3"][:, 0:1],
                    scalar2=None, op0=AOT.is_equal)

                # KBD_g[d128, (h,k16)] = KT_g[d, k0+k16] * Mkf_g[d, (h,k16)]
                kbd = [kbd_big[g][:, st8 * 128:(st8 + 1) * 128] for g in range(2)]

                # V_rep[(rep8,k16), dv256] via replicated-column projection
                vrep_ps = psS.tile([128, 512], FP32, name="vrep", tag="scores")
                nc.tensor.matmul(
                    vrep_ps[:, 0:256], c_sb["rep16"],
                    vnat_st[st8], start=True, stop=True)
                vrep = vrp.tile([128, 256], BF16, name="vrep", tag="vrepsb")
                nc.scalar.activation(vrep, vrep_ps[:, 0:256], Ident)
                # head-masked V sections: vbd[p,(h,dv16)] = vrep[p, dvbase+dv]*Mv[p,h]
                # Wv cols pre-ordered on host: vrep cols = [A(h,dv0-15) | B(h,dv16-31)]
                vbd = []
                for sec in range(2):
                    vb = vrp.tile([128, 128], BF16, name=f"vbd{sec}", tag=f"vbd{sec}")
                    nc.vector.tensor_tensor(
                        out=vb, in0=vrep[:, sec * 128:(sec + 1) * 128],
                        in1=c_sb["mvf"], op=AOT.mult)
                    vbd.append(vb)

                for qc in range(2):
                    q0 = qc * 512
                    ps = psS.tile([128, 512], FP32, name="scores", tag="scores")
                    nc.tensor.matmul(ps, kbd[0], QTs[0][:, q0:q0 + 512],
                                     start=True, stop=False)
                    nc.tensor.matmul(ps, kbd[1], QTs[1][:, q0:q0 + 512],
                                     start=False, stop=False)
                    nc.tensor.matmul(ps, c_sb["lhs1"], g1[:, q0:q0 + 512],
                                     start=False, stop=False)
                    nc.tensor.matmul(ps, c_sb["lhs2"], g2[:, q0:q0 + 512],
                                     start=False, stop=False)
                    nc.tensor.matmul(ps, c_sb["lhs3"], g3[:, q0:q0 + 512],
                                     start=False, stop=True)
                    att = attp.tile([128, 512], BF16, name="att", tag="att")
                    nc.scalar.activation(att, ps, Exp)
                    first, last = (ST == 0), (ST == NST - 1)
                    nc.tensor.matmul(psA[qc], vbd[0], att,
                                     start=first, stop=last,
                                     skip_group_check=True)
                    nc.tensor.matmul(psB[qc], vbd[1], att,
                                     start=first, stop=last,
                                     skip_group_check=True)
                    nc.tensor.matmul(psDD[qc * 32:qc * 32 + 8, :], c_sb["mden"], att,
                                     start=first, stop=last,
                                     skip_group_check=True)

        # ---- normalize + out-projection ----
        for qc in range(2):
            recip = stage.tile([8, 512], FP32, name="recip", tag="recip")
            nc.vector.reciprocal(recip, psDD[qc * 32:qc * 32 + 8, :])
            rb = psS.tile([128, 512], FP32, name="rb", tag="scores")
            nc.tensor.matmul(rb, c_sb["lden"], recip, start=True, stop=True)
            rb_sb = attp.tile([128, 512], FP32, name="rb_sb", tag="rb_sb")
            nc.scalar.copy(rb_sb, rb)
            OA = attp.tile([128, 512], FP16, name="OA", tag="OA")
            OB = attp.tile([128, 512], FP16, name="OB", tag="OB")
            nc.vector.tensor_tensor(out=OA, in0=psA[qc], in1=rb_sb, op=AOT.mult)
            nc.vector.tensor_tensor(out=OB, in0=psB[qc], in1=rb_sb, op=AOT.mult)
            for g in range(2):
                ps = psS.tile([128, 512], FP32, name="fproj", tag="scores")
                nc.tensor.matmul(ps, w_sb["wpA"][:, g * 128:(g + 1) * 128], OA,
                                 start=True, stop=False)
                nc.tensor.matmul(ps, w_sb["wpB"][:, g * 128:(g + 1) * 128], OB,
                                 start=False, stop=True)
                fin = stage.tile([128, 512], FP32, name="fin", tag="fin")
                nc.scalar.activation(fin, ps, Ident, bias=c_sb["bps"][:, g:g + 1])
                nc.sync.dma_start(
                    out=outT[g * 128:(g + 1) * 128, qc * 512:qc * 512 + 512],
                    in_=fin)


_CACHE = {}


def _get_kernel():
    if "nc" not in _CACHE:
        _CACHE["nc"] = _build()
    return _CACHE["nc"]


def _consts(emb_fwd, emb_bwd, Wp, bp, bv):
    """Host-side constant tensors shared across cores."""
    ef = emb_fwd.astype(np.float64)
    eb = emb_bwd.astype(np.float64)
    eye16 = np.eye(16)

    def lhs_for(slots):
        # lhs[(j,k16),(h,k16')] = emb_dir[v_j, h] * [k16==k16']
        nslot = len(slots)
        out = np.zeros((nslot, 16, H, 16), np.float64)
        for j, (dirr, v) in enumerate(slots):
            e = ef if dirr == "F" else eb
            for h in range(H):
                out[j, :, h, :] = e[v, h] * eye16
        return out.reshape(nslot * 16, H * 16).astype(np.float16)

    slots1 = [("F", v) for v in T1V]
    slots2 = [("F", 8)] + [("N", v) for v in T2V[1:]]
    slots3 = [("N", v) for v in T3V]
    lhs1 = lhs_for(slots1)
    lhs2 = lhs_for(slots2)
    lhs3 = lhs_for(slots3)
    vc1 = np.array(T1V, np.float32).repeat(16).reshape(128, 1)
    vc2 = np.array(T2V, np.float32).repeat(16).reshape(128, 1)
    vc3 = np.array(T3V, np.float32).repeat(16).reshape(32, 1)
    didx = np.arange(128)
    mk0 = (didx[:, None] // 32 == np.arange(8)[None, :]).astype(np.float16)
    mk1 = ((didx[:, None] + 128) // 32 == np.arange(8)[None, :]).astype(np.float16)
    mden = (didx[:, None] // 16 == np.arange(8)[None, :]).astype(np.float16)
    lden = mden.T.astype(np.float32).copy()
    import ml_dtypes
    rep16c = np.tile(np.eye(16), (1, 8)).astype(ml_dtypes.bfloat16)
    # rep128[k, (st,h,k16)] = 1[k == st*16 + k16]
    karr = np.arange(128)
    st_i = np.arange(1024) // 128
    k16_i = np.arange(1024) % 16
    rep128c = (karr[:, None] == (st_i * 16 + k16_i)[None, :]).astype(np.float16)
    h_i = (np.arange(1024) // 16) % 8
    mkb0c = (karr[:, None] // 32 == h_i[None, :]).astype(np.float16)
    mkb1c = ((karr[:, None] + 128) // 32 == h_i[None, :]).astype(np.float16)
    mkf0 = np.repeat(mk0, 16, axis=1)
    mkf1 = np.repeat(mk1, 16, axis=1)
    mvf = np.repeat(mden, 16, axis=1)
    # out-proj: Wp rows reordered to (h, dv) A/B sections; bv folded into bp
    WpT = Wp.T.astype(np.float64)  # [dfull, dout]
    rowsA = np.concatenate([np.arange(h * 32, h * 32 + 16) for h in range(H)])
    rowsB = np.concatenate([np.arange(h * 32 + 16, h * 32 + 32) for h in range(H)])
    wpA = WpT[rowsA].astype(np.float16)
    wpB = WpT[rowsB].astype(np.float16)
    bps2 = (bp.astype(np.float64) + Wp.astype(np.float64) @ bv.astype(np.float64))
    bps = np.ascontiguousarray(bps2.reshape(2, 128).T.astype(np.float32))
    return dict(lhs1=lhs1, lhs2=lhs2, lhs3=lhs3, vc1=vc1, vc2=vc2, vc3=vc3,
                mvf=mvf, rep16=rep16c,
                rep128=rep128c, mkb0=mkb0c, mkb1=mkb1c, mden=mden, lden=lden,
                wpA=wpA, wpB=wpB, bps=bps)


def kernel(query, key, value, rel_pos, Wk, bk, Wv, bv, Wq, bq, Wp, bp,
           emb_fwd, emb_bwd):
    query = np.asarray(query, dtype=np.float32)
    key = np.asarray(key, dtype=np.float32)
    value = np.asarray(value, dtype=np.float32)
    rel_pos = np.asarray(rel_pos, dtype=np.int32)
    Wk, Wv, Wq, Wp = (np.asarray(w, dtype=np.float32) for w in (Wk, Wv, Wq, Wp))
    bk, bv, bq, bp = (np.asarray(v, dtype=np.float32) for v in (bk, bv, bq, bp))
    emb_fwd = np.asarray(emb_fwd, dtype=np.float32)
    emb_bwd = np.asarray(emb_bwd, dtype=np.float32)

    gamma = 1.0 / np.sqrt(np.float32(D_K))
    wqT = np.ascontiguousarray((Wq.T * gamma).astype(np.float16))
    wkT = np.ascontiguousarray(Wk.T.astype(np.float16))
    rowsA = np.concatenate([np.arange(h * 32, h * 32 + 16) for h in range(H)])
    rowsB = np.concatenate([np.arange(h * 32 + 16, h * 32 + 32) for h in range(H)])
    wvT = np.ascontiguousarray(Wv.T.astype(np.float16)[:, np.concatenate([rowsA, rowsB])])
    bqs = np.ascontiguousarray((bq * gamma).reshape(2, 128).T)
    bks = np.ascontiguousarray(bk.reshape(2, 128).T)

    consts = _consts(emb_fwd, emb_bwd, Wp, bp, bv)
    nc = _get_kernel()

    import ml_dtypes
    rp_bf = rel_pos.astype(ml_dtypes.bfloat16)

    in_maps = []
    for core in range(N_CORES):
        b, half = divmod(core, 2)
        qs = half * QH
        m = {
            "qT": np.ascontiguousarray(query[b, qs:qs + QH, :].T.astype(np.float16)),
            "kT": np.ascontiguousarray(key[b].T.astype(np.float16)),
            "vT": np.ascontiguousarray(value[b].T.astype(np.float16)),
            "rpF": np.ascontiguousarray(rp_bf[b, qs:qs + QH, :].T),
            "rpN": np.ascontiguousarray(rp_bf[b][:, qs:qs + QH]),
            "wqT": wqT, "wkT": wkT, "wvT": wvT,
            "bqs": bqs, "bks": bks,
        }
        m.update(consts)
        in_maps.append(m)

    global LAST_IN_MAPS
    LAST_IN_MAPS = in_maps
    res = run_bass_kernel_spmd(nc, in_maps, list(range(N_CORES)))

    out = np.empty((B, S, D), dtype=np.float32)
    for core in range(N_CORES):
        b, half = divmod(core, 2)
        qs = half * QH
        out[b, qs:qs + QH, :] = res.results[core]["outT"].T
    return out



# revision 17
# speedup vs baseline: 2.5289x; 2.5289x over previous
"""Trainium2 Bass kernel v3: MultiHeadAttention with rel-pos bias via
host-LUT bias tiles + per-head score layout + PE array tiling.

Problem: B=4, S=2048, D=256, H=8, d_k=32.  8 cores = (batch, query-half);
each core: 8 heads x 1024 q x 2048 k.

v2 (one-hot-plane matmuls in a packed (h,k16) layout) was PE-bound
(~924us MATMUL: 8 matmuls of 512 free per 128x512 score tile) plus 75MB
of 9x-replicated rel_pos DMA (~104 GB/s achieved -> DMA co-critical).

v3: the bias bias[k,q,h] = ef[rpF,h] + eb[rpN,h] takes only 100 values
per head, so the host folds it through a 100x8 LUT into int8 tiles laid
out exactly as the SBUF tiles consume them (16MB/core, contiguous 512KB
DMAs).  Device uses a per-head score layout [128 k, 512 q]:
  scores = K_h^T Q_h / s        1 matmul, contraction 32, row-tiled:
                                4 heads run CONCURRENT in the PE array
                                (tile_position=(32m,0), 4 PSUM banks)
  att_pre = scores + q8         DVE add (int8 bias), out fp16 SBUF
  att     = exp(s*att_pre)      ACT, fp16 in/out (2x rate), scale=s
  psAV   += Vaug_h^T att        1 matmul; Vaug has a ones column so the
                                denominator rides along as out row 32;
                                2 heads/bank at offsets {0,64} run
                                concurrent via col tiling
Per (kt,qc): 2 row-packed KQ spans + 4 col-packed AV spans ~= 2.3us PE,
~3.7us DVE, ~2.6us ACT, ~3.5us DMA -> ~4us/iter * 32 iters.
"""

import sys

if "/opt/trn_rl_repo" not in sys.path:
    sys.path.insert(0, "/opt/trn_rl_repo")

import numpy as np

import concourse.bass as bass
import concourse.mybir as mybir
from concourse import bacc
from concourse.tile import TileContext
from concourse.bass_utils import run_bass_kernel_spmd

B, S, D, H = 4, 2048, 256, 8
D_K = D // H
QH = S // 2
N_CORES = 8
KT = S // 128           # 16 k-tiles of 128
FP32 = mybir.dt.float32
FP16 = mybir.dt.float16
BF16 = mybir.dt.bfloat16
INT8 = mybir.dt.int8

BIAS_INT8 = True        # True: int8 bias tiles (16MB/core); False: fp16 (32MB)
BIAS_DT = INT8 if BIAS_INT8 else FP16


def _build():
    nc = bacc.Bacc("TRN2", target_bir_lowering=False, debug=False)

    qT = nc.dram_tensor("qT", [D, QH], FP16, kind="ExternalInput").ap()
    kT = nc.dram_tensor("kT", [D, S], FP16, kind="ExternalInput").ap()
    vT = nc.dram_tensor("vT", [D, S], FP16, kind="ExternalInput").ap()
    wqT = nc.dram_tensor("wqT", [D, D], FP16, kind="ExternalInput").ap()
    wkT = nc.dram_tensor("wkT", [D, D], FP16, kind="ExternalInput").ap()
    wvT = nc.dram_tensor("wvT", [D, D], FP16, kind="ExternalInput").ap()
    wpT = nc.dram_tensor("wpT", [D, D], FP16, kind="ExternalInput").ap()
    bqs = nc.dram_tensor("bqs", [128, 2], FP32, kind="ExternalInput").ap()
    bps = nc.dram_tensor("bps", [128, 2], FP32, kind="ExternalInput").ap()
    ldn = nc.dram_tensor("ldn", [8, 256], FP16, kind="ExternalInput").ap()
    svec = nc.dram_tensor("svec", [128, 2], FP32, kind="ExternalInput").ap()
    # bias tiles pre-packed host-side: row block (qc*16+kt)*128 .. +128 is
    # one SBUF tile [128 k, (8 h, 512 q)]
    biasT = nc.dram_tensor("biasT", [32 * 128, 8 * 512], BIAS_DT,
                           kind="ExternalInput").ap()
    outT = nc.dram_tensor("outT", [D, QH], FP32, kind="ExternalOutput").ap()

    with TileContext(nc) as tc:
        _emit(nc, tc, locals())
    nc.compile()
    return nc


def _emit(nc, tc, t):
    qT, kT, vT = t["qT"], t["kT"], t["vT"]
    wqT, wkT, wvT, wpT = t["wqT"], t["wkT"], t["wvT"], t["wpT"]
    bqs, bps, ldn, svec = t["bqs"], t["bps"], t["ldn"], t["svec"]
    biasT, outT = t["biasT"], t["outT"]
    Exp = mybir.ActivationFunctionType.Exp
    Ident = mybir.ActivationFunctionType.Identity
    AOT = mybir.AluOpType

    import contextlib
    ctx = contextlib.ExitStack()
    with ctx:
        singles = ctx.enter_context(tc.tile_pool(name="singles", bufs=1))
        stage = ctx.enter_context(tc.tile_pool(name="stage", bufs=2))
        biasp = ctx.enter_context(tc.tile_pool(name="biasp", bufs=3))
        prep = ctx.enter_context(tc.tile_pool(name="prep", bufs=2))
        attp = ctx.enter_context(tc.tile_pool(name="attp", bufs=2))
        # one 4-bank score tile [128, 2048]; all other psum users slice it
        psS = ctx.enter_context(tc.tile_pool(name="psS", bufs=1, space="PSUM"))
        psAV = ctx.enter_context(tc.tile_pool(name="psAV", bufs=1, space="PSUM"))

        # ---- constants ----
        c_sb = {}
        for name, ap, shp, dt in (
            ("bqs", bqs, [128, 2], FP32), ("bps", bps, [128, 2], FP32),
            ("ldn", ldn, [8, 256], FP16), ("svec", svec, [128, 2], FP32),
        ):
            tl = singles.tile(shp, dt, name=name, tag=name)
            nc.sync.dma_start(out=tl, in_=ap)
            c_sb[name] = tl

        # ---- weights: [din-group][128, 256] ----
        w_sb = {}
        for name, ap in (("wq", wqT), ("wk", wkT), ("wv", wvT), ("wp", wpT)):
            for g in range(2):
                tl = singles.tile([128, D], FP16, name=f"w_{name}{g}", tag=f"w_{name}{g}")
                nc.sync.dma_start(out=tl, in_=ap[g * 128:(g + 1) * 128, :])
                w_sb[name, g] = tl

        # ---- raw inputs resident ----
        xin = {}
        for name, ap, width in (("q", qT, QH), ("k", kT, S), ("v", vT, S)):
            for g in range(2):
                tl = singles.tile([128, width], FP16, name=f"{name}in{g}", tag=f"{name}in{g}")
                nc.sync.dma_start(out=tl, in_=ap[g * 128:(g + 1) * 128, :])
                xin[name, g] = tl

        # ---- Q/K projections -> QTs/KTs [g][128, *] fp16 (dout-major) ----
        QTs = [singles.tile([128, QH], FP16, name=f"QTs{g}", tag=f"QTs{g}") for g in range(2)]
        KTs = [singles.tile([128, S], FP16, name=f"KTs{g}", tag=f"KTs{g}") for g in range(2)]
        for dst, src, wname, bias_name, width in (
            (QTs, "q", "wq", "bqs", QH),
            (KTs, "k", "wk", None, S),
        ):
            for c0 in range(0, width, 1024):
                big = psS.tile([128, 2048], FP32, name="proj", tag="scores")
                for ci in range(2):
                    for g in range(2):
                        ps = big[:, (2 * ci + g) * 512:(2 * ci + g) * 512 + 512]
                        cc = c0 + ci * 512
                        for dg in range(2):
                            nc.tensor.matmul(
                                ps, w_sb[wname, dg][:, g * 128:(g + 1) * 128],
                                xin[src, dg][:, cc:cc + 512],
                                start=(dg == 0), stop=(dg == 1))
                        if bias_name:
                            nc.scalar.activation(
                                dst[g][:, cc:cc + 512], ps, Ident,
                                bias=c_sb[bias_name][:, g:g + 1])
                        else:
                            nc.scalar.copy(dst[g][:, cc:cc + 512], ps)

        # ---- Vaug[kt] [128 s, 264=(h: 32 dv + one)] fp16 ----
        vaug = []
        for kt4 in range(KT // 4):
            big = psS.tile([128, 2048], FP32, name="vproj", tag="scores")
            for ki in range(4):
                kt = kt4 * 4 + ki
                vt = singles.tile([128, 264], FP16, name=f"vaug{kt}", tag=f"vaug{kt}")
                ones_ap = bass.AP(tensor=vt.tensor, offset=vt.offset + 32,
                                  ap=[list(vt.ap[0]), [33, 8]])
                nc.gpsimd.memset(ones_ap, 1.0)
                vps = big[:, ki * 512:ki * 512 + 512]
                for dg in range(2):
                    nc.tensor.matmul(
                        vps[:, 0:256], xin["v", dg][:, kt * 128:(kt + 1) * 128],
                        w_sb["wv", dg], start=(dg == 0), stop=(dg == 1))
                dst_ap = bass.AP(tensor=vt.tensor, offset=vt.offset,
                                 ap=[list(vt.ap[0]), [33, 8], [1, 32]])
                src_ap = bass.AP(tensor=vps.tensor, offset=vps.offset,
                                 ap=[list(vps.ap[0]), [32, 8], [1, 32]])
                nc.scalar.copy(dst_ap, src_ap)
                vaug.append(vt)

        # ---- main loop ----
        for qc in range(2):
            q0 = qc * 512
            pav = [psAV.tile([128, 512], FP32, name=f"psAV{j}", tag=f"psAV{j}")
                   for j in range(4)]
            for kt in range(KT):
                bt = biasp.tile([128, 8 * 512], BIAS_DT, name="bt", tag="bt")
                r0 = (qc * KT + kt) * 128
                nc.sync.dma_start(out=bt[:, 0:2048],
                                  in_=biasT[r0:r0 + 128, 0:2048])
                nc.gpsimd.dma_start(out=bt[:, 2048:4096],
                                    in_=biasT[r0:r0 + 128, 2048:4096])
                # per 4-head group: 4 row-packed concurrent KQ matmuls into
                # the bank slices of one 4-bank psum tile, then ONE DVE add
                # and ONE ACT exp over [128, 2048]
                att = []
                for g in range(2):
                    big = psS.tile([128, 2048], FP32, name="scores", tag="scores")
                    for m in range(4):
                        r = 32 * m
                        nc.tensor.matmul(
                            big[:, m * 512:m * 512 + 512],
                            KTs[g][r:r + 32, kt * 128:(kt + 1) * 128],
                            QTs[g][r:r + 32, q0:q0 + 512], start=True, stop=True,
                            tile_position=(r, 0))
                    ap_ = attp.tile([128, 2048], FP16, name="att_pre", tag="att_pre")
                    nc.vector.tensor_tensor(
                        out=ap_, in0=big, in1=bt[:, g * 2048:(g + 1) * 2048],
                        op=AOT.add)
                    # -4 shift keeps exp in fp16 range (logit tail ~12.5 >
                    # ln 65504); cancels between numerator and denominator.
                    at = prep.tile([128, 2048], FP16, name="att", tag="att")
                    nc.scalar.activation(at, ap_, Exp, bias=c_sb["svec"][:, 1:2],
                                         scale=c_sb["svec"][:, 0:1])
                    att.append(at)
                # AV+den: col-packed pairs, 2 heads per PSUM bank at {0, 64}
                for h in range(8):
                    co = 64 * (h % 2)
                    nc.tensor.matmul(
                        pav[h // 2][co:co + 33, :],
                        vaug[kt][:, 33 * h:33 * h + 33],
                        att[h // 4][:, (h % 4) * 512:(h % 4) * 512 + 512],
                        start=(kt == 0), stop=(kt == KT - 1),
                        skip_group_check=True, tile_position=(0, co))

            # ---- normalize + out-projection ----
            # Engines need 32-aligned partition bases, so evacuate den rows
            # (psum rows 32/96 of 4 banks) at their own partitions into 4 col
            # blocks, then one SBUF->SBUF DMA gathers them to 8 partitions.
            # Gathered row order: p = (h%2)*4 + h//2 (row-major over (m, j)).
            denw = stage.tile([128, 4 * 512], FP32, name="denw", tag="denw")
            for h in range(8):
                j, m = h // 2, h % 2
                co = 64 * m + 32
                nc.scalar.copy(denw[co:co + 1, j * 512:(j + 1) * 512],
                               pav[j][co:co + 1, :])
            den = stage.tile([8, 512], FP32, name="den", tag="den")
            nc.sync.dma_start(out=den[0:4, :], in_=denw[32:33, :])
            nc.sync.dma_start(out=den[4:8, :], in_=denw[96:97, :])
            rec = stage.tile([8, 512], FP32, name="rec", tag="rec")
            nc.vector.reciprocal_approx_fast(out=rec, in_=den)
            rec16 = stage.tile([8, 512], FP16, name="rec16", tag="rec16")
            nc.scalar.copy(rec16, rec)
            Og = []
            bigE = psS.tile([128, 2048], FP32, name="endps", tag="scores")
            for g in range(2):
                rb = bigE[:, g * 512:g * 512 + 512]
                nc.tensor.matmul(rb, c_sb["ldn"][:, g * 128:(g + 1) * 128],
                                 rec16, start=True, stop=True)
                rbs = stage.tile([128, 512], FP32, name="rbs", tag="rbs")
                nc.scalar.copy(rbs, rb)
                og = stage.tile([128, 512], FP16, name=f"Og{g}", tag=f"Og{g}")
                for m in range(4):
                    h = 4 * g + m
                    nc.vector.tensor_tensor(
                        out=og[32 * m:32 * m + 32, :],
                        in0=pav[h // 2][64 * (h % 2):64 * (h % 2) + 32, :],
                        in1=rbs[32 * m:32 * m + 32, :], op=AOT.mult)
                Og.append(og)
            for go in range(2):
                ps = bigE[:, (2 + go) * 512:(2 + go) * 512 + 512]
                for gi in range(2):
                    nc.tensor.matmul(
                        ps, w_sb["wp", gi][:, go * 128:(go + 1) * 128], Og[gi],
                        start=(gi == 0), stop=(gi == 1))
                fin = stage.tile([128, 512], FP32, name="fin", tag="fin")
                nc.scalar.activation(fin, ps, Ident, bias=c_sb["bps"][:, go:go + 1])
                nc.sync.dma_start(
                    out=outT[go * 128:(go + 1) * 128, q0:q0 + 512], in_=fin)


_CACHE = {}


def _get_kernel():
    if "nc" not in _CACHE:
        _CACHE["nc"] = _build()
    return _CACHE["nc"]


def prepare_in_maps(query, key, value, rel_pos, Wk, bk, Wv, bv, Wq, bq, Wp, bp,
                    emb_fwd, emb_bwd):
    query = np.asarray(query, dtype=np.float32)
    key = np.asarray(key, dtype=np.float32)
    value = np.asarray(value, dtype=np.float32)
    rel_pos = np.asarray(rel_pos, dtype=np.int32)
    Wk, Wv, Wq, Wp = (np.asarray(w, dtype=np.float32) for w in (Wk, Wv, Wq, Wp))
    bk, bv, bq, bp = (np.asarray(v, dtype=np.float32) for v in (bk, bv, bq, bp))
    emb_fwd = np.asarray(emb_fwd, dtype=np.float32)
    emb_bwd = np.asarray(emb_bwd, dtype=np.float32)

    # 100-entry bias LUT: T2[10*i+j, h] = ef[i,h] + eb[j,h]
    T2 = (emb_fwd[:, None, :] + emb_bwd[None, :, :]).reshape(100, H)
    if BIAS_INT8:
        s = float(max(np.abs(T2).max() / 127.0, 1e-6))
        lut = np.round(T2 / s).astype(np.int8)     # [100, H]
    else:
        s = 1.0
        lut = T2.astype(np.float16)
    lutT = np.ascontiguousarray(lut.T)             # [H, 100]

    gamma = 1.0 / np.sqrt(np.float32(D_K))
    wqT = np.ascontiguousarray((Wq.T * (gamma / s)).astype(np.float16))
    wkT = np.ascontiguousarray(Wk.T.astype(np.float16))
    wvT = np.ascontiguousarray(Wv.T.astype(np.float16))
    wpT = np.ascontiguousarray(Wp.T.astype(np.float16))
    bqs = np.ascontiguousarray((bq * (gamma / s)).reshape(2, 128).T.astype(np.float32))
    # bk is softmax-invariant (adds a per-(h,q) constant across k); dropped.
    # bv folds into bp since softmax rows sum to 1.
    bps2 = bp.astype(np.float64) + Wp.astype(np.float64) @ bv.astype(np.float64)
    bps = np.ascontiguousarray(bps2.reshape(2, 128).T.astype(np.float32))
    # den rows arrive DMA-gathered in order p = (h%2)*4 + h//2
    ldnc = np.zeros((8, 256), np.float16)
    for h in range(H):
        g, m = h // 4, h % 4
        p = (h % 2) * 4 + h // 2
        ldnc[p, g * 128 + 32 * m: g * 128 + 32 * m + 32] = 1.0
    svec = np.stack([np.full(128, s, np.float32),
                     np.full(128, -4.0, np.float32)], axis=1)

    in_maps = []
    for core in range(N_CORES):
        b, half = divmod(core, 2)
        qs = half * QH
        rp = rel_pos[b]
        # bias[h,k,q] = ef[rp[qs+q,k],h] + eb[rp[k,qs+q],h] via LUT on
        # c[k,q] = 10*rp[qs+q,k] + rp[k,qs+q]
        c = rp[qs:qs + QH, :].T * 10 + rp[:, qs:qs + QH]
        bias_hkq = lutT[:, c]                      # [H, S, QH]
        # pack to DMA-tile order: [qc, kt, k(128), h, q(512)]
        bias_dev = np.ascontiguousarray(
            bias_hkq.reshape(H, KT, 128, 2, 512).transpose(3, 1, 2, 0, 4)
        ).reshape(32 * 128, 8 * 512)
        m = {
            "qT": np.ascontiguousarray(query[b, qs:qs + QH, :].T.astype(np.float16)),
            "kT": np.ascontiguousarray(key[b].T.astype(np.float16)),
            "vT": np.ascontiguousarray(value[b].T.astype(np.float16)),
            "wqT": wqT, "wkT": wkT, "wvT": wvT, "wpT": wpT,
            "bqs": bqs, "bps": bps, "ldn": ldnc, "svec": svec,
            "biasT": bias_dev,
        }
        in_maps.append(m)
    return in_maps


def kernel(**inputs):
    nc = _get_kernel()
    in_maps = prepare_in_maps(**inputs)

    global LAST_IN_MAPS
    LAST_IN_MAPS = in_maps
    res = run_bass_kernel_spmd(nc, in_maps, list(range(N_CORES)))

    out = np.empty((B, S, D), dtype=np.float32)
    for core in range(N_CORES):
        b, half = divmod(core, 2)
        qs = half * QH
        out[b, qs:qs + QH, :] = res.results[core]["outT"].T
    return out


# revision 24
# speedup vs baseline: 3.1733x; 1.2548x over previous
"""Trainium2 Bass kernel v3: MultiHeadAttention with rel-pos bias via
host-LUT bias tiles + per-head score layout + PE array tiling.

Problem: B=4, S=2048, D=256, H=8, d_k=32.  8 cores = (batch, query-half);
each core: 8 heads x 1024 q x 2048 k.

v2 (one-hot-plane matmuls in a packed (h,k16) layout) was PE-bound
(~924us MATMUL: 8 matmuls of 512 free per 128x512 score tile) plus 75MB
of 9x-replicated rel_pos DMA (~104 GB/s achieved -> DMA co-critical).

v3: the bias bias[k,q,h] = ef[rpF,h] + eb[rpN,h] takes only 100 values
per head, so the host folds it through a 100x8 LUT into int8 tiles laid
out exactly as the SBUF tiles consume them (16MB/core, contiguous 512KB
DMAs).  Device uses a per-head score layout [128 k, 512 q]:
  scores = K_h^T Q_h / s        1 matmul, contraction 32, row-tiled:
                                4 heads run CONCURRENT in the PE array
                                (tile_position=(32m,0), 4 PSUM banks)
  att_pre = scores + q8         DVE add (int8 bias), out fp16 SBUF
  att     = exp(s*att_pre)      ACT, fp16 in/out (2x rate), scale=s
  psAV   += Vaug_h^T att        1 matmul; Vaug has a ones column so the
                                denominator rides along as out row 32;
                                2 heads/bank at offsets {0,64} run
                                concurrent via col tiling
Per (kt,qc): 2 row-packed KQ spans + 4 col-packed AV spans ~= 2.3us PE,
~3.7us DVE, ~2.6us ACT, ~3.5us DMA -> ~4us/iter * 32 iters.
"""

import sys

if "/opt/trn_rl_repo" not in sys.path:
    sys.path.insert(0, "/opt/trn_rl_repo")

import numpy as np

import concourse.bass as bass
import concourse.mybir as mybir
from concourse import bacc
from concourse.tile import TileContext
from concourse.bass_utils import run_bass_kernel_spmd

B, S, D, H = 4, 2048, 256, 8
D_K = D // H
QH = S // 2
N_CORES = 8
KT = S // 128           # 16 k-tiles of 128
FP32 = mybir.dt.float32
FP16 = mybir.dt.float16
BF16 = mybir.dt.bfloat16
INT8 = mybir.dt.int8

BIAS_INT8 = True        # True: int8 bias tiles (16MB/core); False: fp16 (32MB)
BIAS_DT = INT8 if BIAS_INT8 else FP16


def _build():
    nc = bacc.Bacc("TRN2", target_bir_lowering=False, debug=False)

    qT = nc.dram_tensor("qT", [D, QH], FP16, kind="ExternalInput").ap()
    kT = nc.dram_tensor("kT", [D, S], FP16, kind="ExternalInput").ap()
    vT = nc.dram_tensor("vT", [D, S], FP16, kind="ExternalInput").ap()
    wqT = nc.dram_tensor("wqT", [D, D], FP16, kind="ExternalInput").ap()
    wkT = nc.dram_tensor("wkT", [D, D], FP16, kind="ExternalInput").ap()
    wvT = nc.dram_tensor("wvT", [D, D], FP16, kind="ExternalInput").ap()
    wpT = nc.dram_tensor("wpT", [D, D], FP16, kind="ExternalInput").ap()
    bqs = nc.dram_tensor("bqs", [128, 2], FP32, kind="ExternalInput").ap()
    bps = nc.dram_tensor("bps", [128, 2], FP32, kind="ExternalInput").ap()
    ldn = nc.dram_tensor("ldn", [8, 256], FP16, kind="ExternalInput").ap()
    svec = nc.dram_tensor("svec", [128, 2], FP32, kind="ExternalInput").ap()
    # bias tiles pre-packed host-side: row block (qc*16+kt)*128 .. +128 is
    # one SBUF tile [128 k, (8 h, 512 q)]
    biasT = nc.dram_tensor("biasT", [32 * 128, 8 * 512], BIAS_DT,
                           kind="ExternalInput").ap()
    outT = nc.dram_tensor("outT", [D, QH], FP32, kind="ExternalOutput").ap()

    with TileContext(nc) as tc:
        _emit(nc, tc, locals())
    nc.compile()
    return nc


def _emit(nc, tc, t):
    qT, kT, vT = t["qT"], t["kT"], t["vT"]
    wqT, wkT, wvT, wpT = t["wqT"], t["wkT"], t["wvT"], t["wpT"]
    bqs, bps, ldn, svec = t["bqs"], t["bps"], t["ldn"], t["svec"]
    biasT, outT = t["biasT"], t["outT"]
    Exp = mybir.ActivationFunctionType.Exp
    Ident = mybir.ActivationFunctionType.Identity
    AOT = mybir.AluOpType

    import contextlib
    ctx = contextlib.ExitStack()
    with ctx:
        singles = ctx.enter_context(tc.tile_pool(name="singles", bufs=1))
        stage = ctx.enter_context(tc.tile_pool(name="stage", bufs=2))
        biasp = ctx.enter_context(tc.tile_pool(name="biasp", bufs=3))
        prep = ctx.enter_context(tc.tile_pool(name="prep", bufs=2))
        attp = ctx.enter_context(tc.tile_pool(name="attp", bufs=2))
        # two 2-bank score tiles [128, 1024] rotate so the PE fills one
        # while the DVE drains the other; all other psum users slice them
        psS = ctx.enter_context(tc.tile_pool(name="psS", bufs=2, space="PSUM"))
        psAV = ctx.enter_context(tc.tile_pool(name="psAV", bufs=1, space="PSUM"))

        # ---- constants ----
        c_sb = {}
        for name, ap, shp, dt in (
            ("bqs", bqs, [128, 2], FP32), ("bps", bps, [128, 2], FP32),
            ("ldn", ldn, [8, 256], FP16), ("svec", svec, [128, 2], FP32),
        ):
            tl = singles.tile(shp, dt, name=name, tag=name)
            nc.sync.dma_start(out=tl, in_=ap)
            c_sb[name] = tl

        # ---- weights: [din-group][128, 256] ----
        w_sb = {}
        for name, ap in (("wq", wqT), ("wk", wkT), ("wv", wvT), ("wp", wpT)):
            for g in range(2):
                tl = singles.tile([128, D], FP16, name=f"w_{name}{g}", tag=f"w_{name}{g}")
                nc.sync.dma_start(out=tl, in_=ap[g * 128:(g + 1) * 128, :])
                w_sb[name, g] = tl

        # ---- raw inputs resident ----
        xin = {}
        for name, ap, width in (("q", qT, QH), ("k", kT, S), ("v", vT, S)):
            for g in range(2):
                tl = singles.tile([128, width], FP16, name=f"{name}in{g}", tag=f"{name}in{g}")
                nc.sync.dma_start(out=tl, in_=ap[g * 128:(g + 1) * 128, :])
                xin[name, g] = tl

        # ---- Q/K projections -> QTs/KTs [g][128, *] fp16 (dout-major) ----
        QTs = [singles.tile([128, QH], FP16, name=f"QTs{g}", tag=f"QTs{g}") for g in range(2)]
        KTs = [singles.tile([128, S], FP16, name=f"KTs{g}", tag=f"KTs{g}") for g in range(2)]
        for dst, src, wname, bias_name, width in (
            (QTs, "q", "wq", "bqs", QH),
            (KTs, "k", "wk", None, S),
        ):
            for c0 in range(0, width, 512):
                big = psS.tile([128, 1024], FP32, name="proj", tag="scores")
                for g in range(2):
                    ps = big[:, g * 512:g * 512 + 512]
                    for dg in range(2):
                        nc.tensor.matmul(
                            ps, w_sb[wname, dg][:, g * 128:(g + 1) * 128],
                            xin[src, dg][:, c0:c0 + 512],
                            start=(dg == 0), stop=(dg == 1))
                    if bias_name:
                        nc.scalar.activation(
                            dst[g][:, c0:c0 + 512], ps, Ident,
                            bias=c_sb[bias_name][:, g:g + 1])
                    else:
                        nc.scalar.copy(dst[g][:, c0:c0 + 512], ps)

        # ---- Vaug[kt] [128 s, 264=(h: 32 dv + one)] fp16 ----
        vaug = []
        for kt2 in range(KT // 2):
            big = psS.tile([128, 1024], FP32, name="vproj", tag="scores")
            for ki in range(2):
                kt = kt2 * 2 + ki
                vt = singles.tile([128, 264], FP16, name=f"vaug{kt}", tag=f"vaug{kt}")
                ones_ap = bass.AP(tensor=vt.tensor, offset=vt.offset + 32,
                                  ap=[list(vt.ap[0]), [33, 8]])
                nc.gpsimd.memset(ones_ap, 1.0)
                vps = big[:, ki * 512:ki * 512 + 512]
                for dg in range(2):
                    nc.tensor.matmul(
                        vps[:, 0:256], xin["v", dg][:, kt * 128:(kt + 1) * 128],
                        w_sb["wv", dg], start=(dg == 0), stop=(dg == 1))
                dst_ap = bass.AP(tensor=vt.tensor, offset=vt.offset,
                                 ap=[list(vt.ap[0]), [33, 8], [1, 32]])
                src_ap = bass.AP(tensor=vps.tensor, offset=vps.offset,
                                 ap=[list(vps.ap[0]), [32, 8], [1, 32]])
                nc.scalar.copy(dst_ap, src_ap)
                vaug.append(vt)

        # ---- main loop ----
        for qc in range(2):
            q0 = qc * 512
            pav = [psAV.tile([128, 512], FP32, name=f"psAV{j}", tag=f"psAV{j}")
                   for j in range(4)]
            for kt in range(KT):
                bt = biasp.tile([128, 8 * 512], BIAS_DT, name="bt", tag="bt")
                r0 = (qc * KT + kt) * 128
                for quad in range(4):
                    eng = (nc.sync, nc.gpsimd)[quad % 2]
                    eng.dma_start(
                        out=bt[:, quad * 1024:(quad + 1) * 1024],
                        in_=biasT[r0:r0 + 128, quad * 1024:(quad + 1) * 1024])
                # per head-pair p: 2 row-packed concurrent KQ matmuls into the
                # bank slices of a double-buffered 2-bank psum tile, then ONE
                # DVE add and ONE ACT exp over [128, 1024]
                for p in range(4):
                    g = p // 2
                    big = psS.tile([128, 1024], FP32, name="scores", tag="scores")
                    for m in range(2):
                        r = 32 * ((2 * p + m) % 4)
                        nc.tensor.matmul(
                            big[:, m * 512:m * 512 + 512],
                            KTs[g][r:r + 32, kt * 128:(kt + 1) * 128],
                            QTs[g][r:r + 32, q0:q0 + 512], start=True, stop=True,
                            tile_position=(r, 0))
                    ap_ = attp.tile([128, 1024], FP16, name="att_pre", tag="att_pre")
                    nc.vector.tensor_tensor(
                        out=ap_, in0=big, in1=bt[:, p * 1024:(p + 1) * 1024],
                        op=AOT.add)
                    # -4 shift keeps exp in fp16 range (logit tail ~12.5 >
                    # ln 65504); cancels between numerator and denominator.
                    at = prep.tile([128, 1024], FP16, name="att", tag="att")
                    nc.scalar.activation(at, ap_, Exp, bias=c_sb["svec"][:, 1:2],
                                         scale=c_sb["svec"][:, 0:1])
                    # AV+den: col-packed pair, 2 heads into bank p at {0, 64}
                    for m in range(2):
                        h = 2 * p + m
                        co = 64 * m
                        nc.tensor.matmul(
                            pav[p][co:co + 33, :],
                            vaug[kt][:, 33 * h:33 * h + 33],
                            at[:, m * 512:m * 512 + 512],
                            start=(kt == 0), stop=(kt == KT - 1),
                            skip_group_check=True, tile_position=(0, co))

            # ---- normalize + out-projection ----
            # Engines need 32-aligned partition bases, so evacuate den rows
            # (psum rows 32/96 of 4 banks) at their own partitions into 4 col
            # blocks, then one SBUF->SBUF DMA gathers them to 8 partitions.
            # Gathered row order: p = (h%2)*4 + h//2 (row-major over (m, j)).
            denw = stage.tile([128, 4 * 512], FP32, name="denw", tag="denw")
            for h in range(8):
                j, m = h // 2, h % 2
                co = 64 * m + 32
                nc.scalar.copy(denw[co:co + 1, j * 512:(j + 1) * 512],
                               pav[j][co:co + 1, :])
            den = stage.tile([8, 512], FP32, name="den", tag="den")
            nc.sync.dma_start(out=den[0:4, :], in_=denw[32:33, :])
            nc.sync.dma_start(out=den[4:8, :], in_=denw[96:97, :])
            rec = stage.tile([8, 512], FP32, name="rec", tag="rec")
            nc.vector.reciprocal_approx_fast(out=rec, in_=den)
            rec16 = stage.tile([8, 512], FP16, name="rec16", tag="rec16")
            nc.scalar.copy(rec16, rec)
            Og = []
            bigE = psS.tile([128, 1024], FP32, name="endps", tag="scores")
            bigE2 = psS.tile([128, 1024], FP32, name="endps2", tag="scores")
            for g in range(2):
                rb = bigE[:, g * 512:g * 512 + 512]
                nc.tensor.matmul(rb, c_sb["ldn"][:, g * 128:(g + 1) * 128],
                                 rec16, start=True, stop=True)
                rbs = stage.tile([128, 512], FP32, name="rbs", tag="rbs")
                nc.scalar.copy(rbs, rb)
                og = stage.tile([128, 512], FP16, name=f"Og{g}", tag=f"Og{g}")
                for m in range(4):
                    h = 4 * g + m
                    nc.vector.tensor_tensor(
                        out=og[32 * m:32 * m + 32, :],
                        in0=pav[h // 2][64 * (h % 2):64 * (h % 2) + 32, :],
                        in1=rbs[32 * m:32 * m + 32, :], op=AOT.mult)
                Og.append(og)
            for go in range(2):
                ps = bigE2[:, go * 512:go * 512 + 512]
                for gi in range(2):
                    nc.tensor.matmul(
                        ps, w_sb["wp", gi][:, go * 128:(go + 1) * 128], Og[gi],
                        start=(gi == 0), stop=(gi == 1))
                fin = stage.tile([128, 512], FP32, name="fin", tag="fin")
                nc.scalar.activation(fin, ps, Ident, bias=c_sb["bps"][:, go:go + 1])
                nc.sync.dma_start(
                    out=outT[go * 128:(go + 1) * 128, q0:q0 + 512], in_=fin)


_CACHE = {}


def _get_kernel():
    if "nc" not in _CACHE:
        _CACHE["nc"] = _build()
    return _CACHE["nc"]


def prepare_in_maps(query, key, value, rel_pos, Wk, bk, Wv, bv, Wq, bq, Wp, bp,
                    emb_fwd, emb_bwd):
    query = np.asarray(query, dtype=np.float32)
    key = np.asarray(key, dtype=np.float32)
    value = np.asarray(value, dtype=np.float32)
    rel_pos = np.asarray(rel_pos, dtype=np.int32)
    Wk, Wv, Wq, Wp = (np.asarray(w, dtype=np.float32) for w in (Wk, Wv, Wq, Wp))
    bk, bv, bq, bp = (np.asarray(v, dtype=np.float32) for v in (bk, bv, bq, bp))
    emb_fwd = np.asarray(emb_fwd, dtype=np.float32)
    emb_bwd = np.asarray(emb_bwd, dtype=np.float32)

    # 100-entry bias LUT: T2[10*i+j, h] = ef[i,h] + eb[j,h]
    T2 = (emb_fwd[:, None, :] + emb_bwd[None, :, :]).reshape(100, H)
    if BIAS_INT8:
        s = float(max(np.abs(T2).max() / 127.0, 1e-6))
        lut = np.round(T2 / s).astype(np.int8)     # [100, H]
    else:
        s = 1.0
        lut = T2.astype(np.float16)
    lutT = np.ascontiguousarray(lut.T)             # [H, 100]

    gamma = 1.0 / np.sqrt(np.float32(D_K))
    wqT = np.ascontiguousarray((Wq.T * (gamma / s)).astype(np.float16))
    wkT = np.ascontiguousarray(Wk.T.astype(np.float16))
    wvT = np.ascontiguousarray(Wv.T.astype(np.float16))
    wpT = np.ascontiguousarray(Wp.T.astype(np.float16))
    bqs = np.ascontiguousarray((bq * (gamma / s)).reshape(2, 128).T.astype(np.float32))
    # bk is softmax-invariant (adds a per-(h,q) constant across k); dropped.
    # bv folds into bp since softmax rows sum to 1.
    bps2 = bp.astype(np.float64) + Wp.astype(np.float64) @ bv.astype(np.float64)
    bps = np.ascontiguousarray(bps2.reshape(2, 128).T.astype(np.float32))
    # den rows arrive DMA-gathered in order p = (h%2)*4 + h//2
    ldnc = np.zeros((8, 256), np.float16)
    for h in range(H):
        g, m = h // 4, h % 4
        p = (h % 2) * 4 + h // 2
        ldnc[p, g * 128 + 32 * m: g * 128 + 32 * m + 32] = 1.0
    svec = np.stack([np.full(128, s, np.float32),
                     np.full(128, -4.0, np.float32)], axis=1)

    in_maps = []
    for core in range(N_CORES):
        b, half = divmod(core, 2)
        qs = half * QH
        rp = rel_pos[b]
        # bias[h,k,q] = ef[rp[qs+q,k],h] + eb[rp[k,qs+q],h] via LUT on
        # c[k,q] = 10*rp[qs+q,k] + rp[k,qs+q]
        c = rp[qs:qs + QH, :].T * 10 + rp[:, qs:qs + QH]
        bias_hkq = lutT[:, c]                      # [H, S, QH]
        # pack to DMA-tile order: [qc, kt, k(128), h, q(512)]
        bias_dev = np.ascontiguousarray(
            bias_hkq.reshape(H, KT, 128, 2, 512).transpose(3, 1, 2, 0, 4)
        ).reshape(32 * 128, 8 * 512)
        m = {
            "qT": np.ascontiguousarray(query[b, qs:qs + QH, :].T.astype(np.float16)),
            "kT": np.ascontiguousarray(key[b].T.astype(np.float16)),
            "vT": np.ascontiguousarray(value[b].T.astype(np.float16)),
            "wqT": wqT, "wkT": wkT, "wvT": wvT, "wpT": wpT,
            "bqs": bqs, "bps": bps, "ldn": ldnc, "svec": svec,
            "biasT": bias_dev,
        }
        in_maps.append(m)
    return in_maps


def kernel(**inputs):
    nc = _get_kernel()
    in_maps = prepare_in_maps(**inputs)

    global LAST_IN_MAPS
    LAST_IN_MAPS = in_maps
    res = run_bass_kernel_spmd(nc, in_maps, list(range(N_CORES)))

    out = np.empty((B, S, D), dtype=np.float32)
    for core in range(N_CORES):
        b, half = divmod(core, 2)
        qs = half * QH
        out[b, qs:qs + QH, :] = res.results[core]["outT"].T
    return out
